# revision 9
# baseline (speedup 1.0000x reference)
"""Distributed HGNN+ convolution for 8 Trainium2 NeuronCores (Bass/Tile).

Math (dense hypergraph incidence H [N_V, N_E], features X [N_V, C]):
    Xt  = X @ W.T + b                    # theta
    Xe  = (H.T @ Xt) * 1/colsum(H)       # V2E mean aggregation
    Xv  = (H @ Xe)   * 1/rowsum(H)       # E2V mean aggregation
    out = relu(Xv)

Distribution: vertex rows are sharded across the 8 cores: each core
computes theta on its vertex shard, a partial V2E GEMM, a chunked bf16
AllReduce of the partial edge features (chunks overlap the V2E GEMM
tail), then a fully row-parallel E2V GEMM over its own vertex rows.

The degree scalings are folded in on the host: 1/colsum(H) is baked
into the E2V incidence panels (htp = H/d_e) and 1/rowsum(H) ships as a
tiny per-vertex vector applied as the activation scale of the final
ReLU. This removes the on-device degree computation entirely and keeps
every GEMM a clean 512-wide single-PSUM-bank matmul.

DMA layouts are chosen so every large transfer moves >=16KB contiguous
per partition (4KB-line transfers measured only ~140GB/s aggregate vs
~360GB/s for 16KB lines): V2E incidence panels are packed 4 edge-tiles
wide, theta inputs are pre-transposed to partition-major, and V2E
partial outputs are batched 4 tiles per DRAM store.

Compute is bf16 with fp32 PSUM accumulation (well within the 2e-2
relative-error envelope); the all-reduce is bf16.
"""

import contextlib

import numpy as np
import ml_dtypes

BF16 = ml_dtypes.bfloat16

# Problem shape (hardcoded per contract).
N_V, N_E, CH, NCORES = 16384, 8192, 512, 8


def _full_cfg():
    # AllReduce chunk sizes in 128-edge tiles (must each divide by 4).
    # Front chunks are big (amortize per-collective overhead while V2E
    # compute hides them); the tail chunks are small so the last AR +
    # gather hides under the held-pair E2V pre-work.
    return dict(n_v=N_V, n_e=N_E, ch=CH, ncores=NCORES,
                chunks=(16, 16, 16, 8, 8))


def build_graph(tc, io, cfg):
    """Emit the Tile IR. io: dict of DRAM APs: hsp, htp, xta, wtb, dvr, out."""
    from concourse import mybir

    nc = tc.nc
    f32 = mybir.dt.float32
    bf16 = mybir.dt.bfloat16
    Relu = mybir.ActivationFunctionType.Relu

    n_v, n_e, ch, ncores = cfg["n_v"], cfg["n_e"], cfg["ch"], cfg["ncores"]
    chunks = list(cfg["chunks"])    # AllReduce chunk sizes in edge tiles
    nch = len(chunks)
    VS = n_v // ncores      # vertices per core
    KV = VS // 128          # vertex 128-tiles per core
    EM = n_e // 128         # edge 128-tiles (global)
    CKT = ch // 128 + 1     # theta contraction tiles (in-channels + bias row)
    G4 = 4                  # edge tiles per V2E panel / store group
    NG = EM // G4           # V2E panel groups
    assert sum(chunks) == EM and all(c % G4 == 0 for c in chunks)
    cstart = [sum(chunks[:c]) for c in range(nch)]   # chunk start (edge tiles)
    # group index -> (chunk, last-group-of-chunk?)
    g2c = {}
    for c in range(nch):
        for jg in range(chunks[c] // G4):
            g = (cstart[c] + jg * G4) // G4
            g2c[g] = (c, jg == chunks[c] // G4 - 1)
    rg = [list(range(ncores))]

    hsp, htp, xta, wtb, dvr, out = (
        io["hsp"], io["htp"], io["xta"], io["wtb"], io["dvr"], io["out"],
    )

    with contextlib.ExitStack() as ctx:
        theta_in = ctx.enter_context(tc.tile_pool(name="theta_in", bufs=1))
        xt_pool = ctx.enter_context(tc.tile_pool(name="xt_pool", bufs=1))
        xe_pool = ctx.enter_context(tc.tile_pool(name="xe_pool", bufs=1))
        hs_pool = ctx.enter_context(tc.tile_pool(name="hs_pool", bufs=2))
        ht_pool = ctx.enter_context(tc.tile_pool(name="ht_pool", bufs=3))
        sb_out = ctx.enter_context(tc.tile_pool(name="sb_out", bufs=3))
        psum = ctx.enter_context(tc.tile_pool(name="psum", bufs=2, space="PSUM"))
        dram = ctx.enter_context(tc.tile_pool(name="dram", bufs=1, space="DRAM"))

        # ---- theta: Xt = [X | 1] @ [W.T ; b], kept in SBUF as KV tiles of
        # [128 v, ch]. Inputs come pre-transposed partition-major so the
        # loads are two single big-line DMAs.
        xta_sb = theta_in.tile([128, CKT * VS], bf16)
        nc.sync.dma_start(xta_sb, xta)
        wtb_sb = theta_in.tile([128, CKT * ch], bf16)
        nc.sync.dma_start(wtb_sb, wtb)
        dvr_sb = theta_in.tile([128, KV], f32)
        nc.sync.dma_start(dvr_sb, dvr)

        xt_all = xt_pool.tile([128, KV * ch], bf16)

        for vm in range(KV):
            ps = psum.tile([128, ch], f32, tag="ps", bufs=4, name="ps_theta")
            for kt in range(CKT):
                nc.tensor.matmul(
                    ps,
                    lhsT=xta_sb[:, kt * VS + vm * 128 : kt * VS + (vm + 1) * 128],
                    rhs=wtb_sb[:, kt * ch : (kt + 1) * ch],
                    start=(kt == 0),
                    stop=(kt == CKT - 1),
                )
            nc.vector.tensor_copy(xt_all[:, vm * ch : (vm + 1) * ch], ps)

        # ---- V2E partial GEMM + chunked AllReduce.
        # DMA issue-engine assignment matters: each engine's instruction
        # stream is serial, and a DMA issue that waits (e.g. the xe gather
        # waiting on its AllReduce) blocks every later issue on that engine.
        # So: hs/ht panel loads + xe gathers on sync (gathers emitted after
        # all hs loads, interleaved with the first ht loads so a pending
        # gather never delays a panel the tensor engine needs sooner);
        # arin stores + output stores on scalar; AR triggers on gpsimd.
        arin = [
            dram.tile([128, chunks[c] * ch], bf16, name=f"arin{c}",
                      tag=f"arin{c}")
            for c in range(nch)
        ]
        arout = [
            dram.tile([128, chunks[c] * ch], bf16, name=f"arout{c}",
                      tag=f"arout{c}", addr_space="Shared")
            for c in range(nch)
        ]
        xe_all = xe_pool.tile([128, EM * ch], bf16)

        def gather(c):
            nc.sync.dma_start(
                xe_all[:, cstart[c] * ch : (cstart[c] + chunks[c]) * ch],
                arout[c],
            )

        for g4 in range(NG):
            hs_sb = hs_pool.tile([128, KV * G4 * 128], bf16, tag="hs", name="hs_sb")
            nc.sync.dma_start(hs_sb, hsp[g4])
            ar_sb = sb_out.tile([128, G4 * ch], bf16, tag="ar_sb", bufs=2,
                                name="ar_sb")
            for g in range(G4):
                ps = psum.tile([128, ch], f32, tag="ps", bufs=4, name="ps_v2e")
                for kt in range(KV):
                    nc.tensor.matmul(
                        ps,
                        lhsT=hs_sb[:, kt * 512 + g * 128 : kt * 512 + (g + 1) * 128],
                        rhs=xt_all[:, kt * ch : (kt + 1) * ch],
                        start=(kt == 0),
                        stop=(kt == KV - 1),
                    )
                nc.vector.tensor_copy(ar_sb[:, g * ch : (g + 1) * ch], ps)
            c, last = g2c[g4]
            jg = g4 - cstart[c] // G4
            nc.scalar.dma_start(
                arin[c][:, jg * G4 * ch : (jg + 1) * G4 * ch], ar_sb
            )
            if last:
                nc.gpsimd.collective_compute(
                    "AllReduce",
                    mybir.AluOpType.add,
                    replica_groups=rg,
                    ins=[arin[c].opt()],
                    outs=[arout[c].opt()],
                )
        # Gathers for all but the last two chunks: their ARs complete
        # during V2E, so these issues never block the ht loads behind them.
        for c in range(max(0, nch - 2)):
            gather(c)

        # ---- E2V GEMM (row-parallel, incidence pre-scaled by 1/d_e) +
        # rowsum scaling via activation scale + ReLU. The first two vertex
        # tiles accumulate all but the last two chunks first, so their
        # matmuls overlap the tail AllReduces; everything later streams at
        # full rate. (The pre-work may only cover chunks whose gathers are
        # emitted BEFORE it — program order defines the dataflow.)
        KE3 = cstart[nch - 2] if nch > 2 else 0

        def e2v_mm(ps, ht_sb, ke, start, stop):
            nc.tensor.matmul(
                ps,
                lhsT=ht_sb[:, ke * 128 : (ke + 1) * 128],
                rhs=xe_all[:, ke * ch : (ke + 1) * ch],
                start=start,
                stop=stop,
            )

        def e2v_tail(vm, ps, ht_sb, ke0):
            for ke in range(ke0, EM):
                e2v_mm(ps, ht_sb, ke, ke == 0, ke == EM - 1)
            o_sb = sb_out.tile([128, ch], f32, tag="o_sb", bufs=2, name="o_sb")
            nc.scalar.activation(o_sb, ps, Relu, scale=dvr_sb[:, vm : vm + 1])
            nc.scalar.dma_start(out[vm * 128 : (vm + 1) * 128, :], o_sb)

        nsplit = 2 if nch > 1 else 0
        held = []
        for vm in range(nsplit):
            ht_sb = ht_pool.tile([128, EM * 128], bf16, tag="ht", name="ht_sb")
            nc.sync.dma_start(ht_sb, htp[vm])
            ps = psum.tile([128, ch], f32, tag="pse", bufs=3, name="ps_e2v")
            for ke in range(KE3):
                e2v_mm(ps, ht_sb, ke, ke == 0, False)
            held.append((vm, ps, ht_sb))
        # Last two gathers issue after the first ht loads: their waits
        # (on the tail ARs) overlap the held-pair pre-work above.
        for c in range(max(0, nch - 2), nch):
            gather(c)
        for vm, ps, ht_sb in held:
            e2v_tail(vm, ps, ht_sb, KE3)
        for vm in range(nsplit, KV):
            ht_sb = ht_pool.tile([128, EM * 128], bf16, tag="ht", name="ht_sb")
            nc.sync.dma_start(ht_sb, htp[vm])
            ps = psum.tile([128, ch], f32, tag="pse", bufs=3, name="ps_e2v")
            e2v_tail(vm, ps, ht_sb, 0)


def pack_inputs(X, H, W, b, cfg):
    """Host-side shard/cast/pack. Returns one input map per core."""
    from concurrent.futures import ThreadPoolExecutor

    n_v, n_e, ch, ncores = cfg["n_v"], cfg["n_e"], cfg["ch"], cfg["ncores"]
    VS = n_v // ncores
    KV = VS // 128
    EM = n_e // 128
    CKT = ch // 128 + 1
    G4 = 4
    NG = EM // G4

    # Degree scalings, computed once in f32 on the full H.
    d_e = H.sum(axis=0)
    d_v = H.sum(axis=1)
    de_r = np.where(d_e == 0, 0, 1.0 / d_e).astype(np.float32)
    dv_r = np.where(d_v == 0, 0, 1.0 / d_v).astype(np.float32)

    wtb_rows = np.vstack(
        [
            np.ascontiguousarray(W.T).astype(np.float32),
            b[None, :].astype(np.float32),
            np.zeros((127, ch), np.float32),
        ]
    ).astype(BF16)
    # partition-major: wtb[p, kt*ch + f] = wtb_rows[kt*128 + p, f]
    wtb = np.ascontiguousarray(
        wtb_rows.reshape(CKT, 128, ch).transpose(1, 0, 2).reshape(128, CKT * ch)
    )

    def pack_core(c):
        Hc = H[c * VS : (c + 1) * VS]
        Hc_bf = Hc.astype(BF16)
        # hsp[g4, p, kt*512 + g*128 + f] = Hc[kt*128+p, (g4*4+g)*128+f]
        R = Hc_bf.reshape(KV, 128, NG, G4, 128)
        hsp = np.ascontiguousarray(R.transpose(2, 1, 0, 3, 4)).reshape(
            NG, 128, KV * G4 * 128
        )
        # htp[vm, p, ke*128+f] = (Hc/d_e)[vm*128+f, ke*128+p]  (E2V lhsT)
        Hs = (Hc * de_r[None, :]).astype(BF16)
        R2 = Hs.reshape(KV, 128, EM, 128)
        htp = np.ascontiguousarray(R2.transpose(0, 3, 2, 1)).reshape(KV, 128, n_e)
        Xc = X[c * VS : (c + 1) * VS]
        xta_rows = np.vstack(
            [
                np.ascontiguousarray(Xc.T),
                np.ones((1, VS), np.float32),
                np.zeros((127, VS), np.float32),
            ]
        ).astype(BF16)
        xta = np.ascontiguousarray(
            xta_rows.reshape(CKT, 128, VS).transpose(1, 0, 2).reshape(128, CKT * VS)
        )
        # dvr[p, vm] = 1/d_v[c*VS + vm*128 + p]
        dvr = np.ascontiguousarray(
            dv_r[c * VS : (c + 1) * VS].reshape(KV, 128).T
        )
        return dict(hsp=hsp, htp=htp, xta=xta, wtb=wtb, dvr=dvr)

    with ThreadPoolExecutor(max_workers=ncores) as ex:
        return list(ex.map(pack_core, range(ncores)))


_cache = {}


def _build_compiled(cfg, reps=1):
    key = (tuple(sorted(cfg.items())), reps)
    if key in _cache:
        return _cache[key]
    from concourse import bacc, mybir, tile

    n_v, n_e, ch, ncores = cfg["n_v"], cfg["n_e"], cfg["ch"], cfg["ncores"]
    VS = n_v // ncores
    KV = VS // 128
    EM = n_e // 128
    CKT = ch // 128 + 1
    G4 = 4
    NG = EM // G4

    nc = bacc.Bacc("TRN2", target_bir_lowering=False, debug=False,
                   num_devices=ncores)
    io = {
        "hsp": nc.dram_tensor("hsp", [NG, 128, KV * G4 * 128], mybir.dt.bfloat16,
                              kind="ExternalInput").ap(),
        "htp": nc.dram_tensor("htp", [KV, 128, n_e], mybir.dt.bfloat16,
                              kind="ExternalInput").ap(),
        "xta": nc.dram_tensor("xta", [128, CKT * VS], mybir.dt.bfloat16,
                              kind="ExternalInput").ap(),
        "wtb": nc.dram_tensor("wtb", [128, CKT * ch], mybir.dt.bfloat16,
                              kind="ExternalInput").ap(),
        "dvr": nc.dram_tensor("dvr", [128, KV], mybir.dt.float32,
                              kind="ExternalInput").ap(),
        "out": nc.dram_tensor("out", [VS, ch], mybir.dt.float32,
                              kind="ExternalOutput").ap(),
    }
    with tile.TileContext(nc) as tc:
        for _ in range(reps):
            build_graph(tc, io, cfg)
    nc.compile()
    _cache[key] = nc
    return nc


def kernel(X, H, W, b, _trace=False, _cfg=None):
    from concourse.bass_utils import run_bass_kernel_spmd

    cfg = _cfg or _full_cfg()
    X = np.asarray(X, dtype=np.float32)
    H = np.asarray(H, dtype=np.float32)
    W = np.asarray(W, dtype=np.float32)
    b = np.asarray(b, dtype=np.float32)

    nc = _build_compiled(cfg)
    in_maps = pack_inputs(X, H, W, b, cfg)
    res = run_bass_kernel_spmd(
        nc, in_maps, core_ids=list(range(cfg["ncores"])), trace=_trace
    )
    kernel.last_result = res
    return np.concatenate([r["out"] for r in res.results], axis=0)


kernel.last_result = None
